# revision 28
# baseline (speedup 1.0000x reference)
"""Multi-head causal attention on 8 Trainium2 NeuronCores.

Sharding: core c -> (batch b = c//2, head-half hh = c%2).  Each core computes
q/k/v projections for its 8 heads (column-sharded wq/wk/wv), causal attention,
and a full-width partial output projection (row-sharded wo).  Host sums the
two partials per batch and adds the bias.

Device-side layout trick: scores are computed transposed (scoresT[j, i]) so
that the softmax-weighted sum over keys (ctx) is a plain matmul with v as the
stationary operand.  Ones-columns baked alongside v produce the softmax
denominator replicated across 64 partitions in the same PSUM tile as ctx.

Scheduling: projection chunks (per 512-token s-block) and output-projection
tiles are emitted as PE "filler" interleaved into the attention jt-loop, so
the tensor engine stays busy while the scalar engine paces the exp chain.
The two heads of a pair run as row-tiled concurrent K=64 matmuls (partitions
0:64 / 64:128), so a score pair costs one 512-column pass.
"""

import numpy as np

import concourse.bass as bass
import concourse.mybir as mybir
import concourse.tile as tile
from concourse import bacc
from concourse.bass_utils import run_bass_kernel_spmd

# Problem shape (hardcoded; kernel.py must be self-contained).
B, S, D, H = 4, 2048, 1024, 16
HD = D // H           # 64 head dim
NCORES = 8
EH = D // 2           # 512: per-core e-width (8 heads)
NHL = H // 2          # 8 local heads per core
SB = 512              # s-block (free dim of most matmuls)
NSB = S // SB         # 4
NST = S // 128        # 16 s-tiles / j-tiles
NEG = EH // 128       # 4 e-groups of 128 partitions
NKG = D // 128        # 8 d-groups (contraction tiles)
VROW = 4 * 192        # v_ext row: 4x [v_even(64) | ones(64) | v_odd(64)] = 768

F32 = mybir.dt.float32
F32R = mybir.dt.float32r
BF16 = mybir.dt.bfloat16
MMDT = BF16          # dtype for matmul inputs (BF16 or F32R)
import ml_dtypes
MMNP = ml_dtypes.bfloat16 if MMDT == BF16 else np.float32

TRACE = False
LAST_RESULT = None


def _build():
    nc = bacc.Bacc()

    xT_d = nc.dram_tensor("xt", [NSB, NKG, 128, SB], MMDT, kind="ExternalInput")
    wqT_d = nc.dram_tensor("wqt", [128, NEG * NKG * 128], MMDT, kind="ExternalInput")
    wkT_d = nc.dram_tensor("wkt", [128, NEG * NKG * 128], MMDT, kind="ExternalInput")
    wvT_d = nc.dram_tensor("wvt", [128, NKG * EH], MMDT, kind="ExternalInput")
    woT_d = nc.dram_tensor("wot", [128, NEG * D], MMDT, kind="ExternalInput")
    masks_d = nc.dram_tensor("masks", [128, 128], MMDT, kind="ExternalInput")
    out_d = nc.dram_tensor("out", [S, D], F32, kind="ExternalOutput")

    with tile.TileContext(nc) as tc:
        with (
            tc.tile_pool(name="persist", bufs=1) as persist,
            tc.tile_pool(name="sharedp", bufs=2, space="PSUM") as sharedp,
            tc.tile_pool(name="ctxps", bufs=2, space="PSUM") as ctxps,
            tc.tile_pool(name="sp", bufs=2, space="PSUM") as sp,
            tc.tile_pool(name="expp", bufs=4) as expp,
            tc.tile_pool(name="smallp", bufs=5) as smallp,
            tc.tile_pool(name="p3", bufs=2) as p3,
        ):
            qT = persist.tile([128, NEG, S], MMDT)      # [e-part, e-group, s]
            kT = persist.tile([128, NEG, S], MMDT)
            v_ext = persist.tile([128, NST, VROW], MMDT)  # [s-part, s-tile, row]
            ctxT = persist.tile([128, NEG, S], MMDT)
            w_q = persist.tile([128, NEG, NKG, 128], MMDT)
            w_k = persist.tile([128, NEG, NKG, 128], MMDT)
            w_v = persist.tile([128, NKG, EH], MMDT)
            woT_sb = persist.tile([128, NEG, D], MMDT)
            masks_sb = persist.tile([128, 128], MMDT)
            xts_all = persist.tile([128, NSB, NKG, SB], MMDT)

            # input DMAs: host pre-arranges every tensor into the exact
            # SBUF layout, so each transfer is contiguous on both sides;
            # w_q/w_k/x are split finely so chunk-0 work starts early
            WCH = NKG * 128
            for mt in range(NEG):
                nc.gpsimd.dma_start(
                    out=w_q[:, mt, :, :],
                    in_=wqT_d[:, mt * WCH : (mt + 1) * WCH].rearrange(
                        "p (a e) -> p a e", a=NKG
                    ),
                )
            for kg in range(NKG):
                nc.sync.dma_start(
                    out=xts_all[:, 0, kg, :], in_=xT_d[0, kg]
                )
            for mt in range(NEG):
                nc.gpsimd.dma_start(
                    out=w_k[:, mt, :, :],
                    in_=wkT_d[:, mt * WCH : (mt + 1) * WCH].rearrange(
                        "p (a e) -> p a e", a=NKG
                    ),
                )
            for sb in range(1, NSB):
                for kg in range(NKG):
                    nc.sync.dma_start(
                        out=xts_all[:, sb, kg, :], in_=xT_d[sb, kg]
                    )
            nc.scalar.dma_start(
                out=w_v, in_=wvT_d[:, :].rearrange("p (a e) -> p a e", a=NKG)
            )
            nc.scalar.dma_start(out=masks_sb, in_=masks_d[:, :])
            nc.scalar.dma_start(
                out=woT_sb,
                in_=woT_d[:, :].rearrange("p (a e) -> p a e", a=NEG),
            )

            # shared ones block between each (even, odd) head pair
            for st in range(NST):
                for p in range(4):
                    ones_ap = v_ext[:, st, p * 192 + 64 : p * 192 + 128]
                    if MMDT == F32R:
                        ones_ap = ones_ap.bitcast(F32)
                    nc.vector.memset(ones_ap, 1.0)

            # ---------------- filler work generators ----------------
            def qk_closure(w_sb, dst, mt, sb):
                def go():
                    ssl = slice(sb * SB, (sb + 1) * SB)
                    msl = slice(mt * 128, (mt + 1) * 128)
                    ps = sharedp.tile([128, SB], F32, tag="acc")
                    for kg in range(NKG):
                        nc.tensor.matmul(
                            out=ps,
                            lhsT=(w_sb[:, mt, kg, :]),
                            rhs=(xts_all[:, sb, kg, :]),
                            start=(kg == 0),
                            stop=(kg == NKG - 1),
                        )
                    nc.vector.tensor_copy(dst[:, mt, ssl], ps)
                return go

            def v_closure(st4, sb):
                def go():
                    st = sb * (SB // 128) + st4
                    ps = sharedp.tile([128, EH], F32, tag="acc")
                    xsl = slice(st4 * 128, (st4 + 1) * 128)
                    for kg in range(NKG):
                        nc.tensor.matmul(
                            out=ps,
                            lhsT=(xts_all[:, sb, kg, xsl]),
                            rhs=(w_v[:, kg, :]),
                            start=(kg == 0),
                            stop=(kg == NKG - 1),
                        )
                    # psum cols: head h at [h*64, h*64+64); dest pair p:
                    # even head -> p*192+128, odd head -> p*192
                    psr = ps[:].rearrange("p (a c) -> p a c", c=128)
                    vst = v_ext[:, st, :].rearrange("p (a w) -> p a w", w=192)
                    nc.vector.tensor_copy(vst[:, :, 128:192], psr[:, :, 0:64])
                    nc.vector.tensor_copy(vst[:, :, 0:64], psr[:, :, 64:128])
                return go

            def chunk_closures(sb):
                cl = []
                for w_sb, dst in ((w_q, qT), (w_k, kT)):
                    for mt in range(NEG):
                        cl.append(qk_closure(w_sb, dst, mt, sb))
                for st4 in range(SB // 128):
                    cl.append(v_closure(st4, sb))
                return cl

            def p3_closure(it, ob):
                def go():
                    itsl = slice(it * 128, (it + 1) * 128)
                    osl = slice(ob * SB, (ob + 1) * SB)
                    ps = sharedp.tile([128, SB], F32, tag="acc")
                    for gg in range(NEG):
                        nc.tensor.matmul(
                            out=ps,
                            lhsT=(ctxT[:, gg, itsl]),
                            rhs=(woT_sb[:, gg, osl]),
                            start=(gg == 0),
                            stop=(gg == NEG - 1),
                        )
                    ot = p3.tile([128, SB], F32, tag="ot")
                    nc.vector.tensor_copy(ot, ps)
                    nc.sync.dma_start(out=out_d[itsl, osl], in_=ot)
                return go

            def p3_closures(ib):
                return [
                    p3_closure(it, ob)
                    for it in range(4 * ib, 4 * ib + 4)
                    for ob in range(2)
                ]

            # ---------------- attention with interleaved filler ----------------
            # i-block order [0,1,3,2]: the exp-heaviest block (3) runs while
            # p3 filler is still available, and the tail drains on the
            # smaller block 2 (whose own p3 is then the only serial tail)
            ib_order = [0, 1, 3, 2]
            must = list(chunk_closures(0))  # chunk 0 must precede attention 0
            for cl in must:
                cl()
            must = []
            opt = []
            chunks_done = {0}

            for idx, ib in enumerate(ib_order):
                isl = slice(ib * SB, (ib + 1) * SB)
                njt = 4 * (ib + 1)
                if idx + 1 < len(ib_order):
                    must = [
                        cl
                        for c in range(ib_order[idx + 1] + 1)
                        if c not in chunks_done
                        for cl in chunk_closures(c)
                    ]
                    chunks_done.update(range(ib_order[idx + 1] + 1))
                if idx >= 1:
                    opt.extend(p3_closures(ib_order[idx - 1]))
                steps_left = 2 * njt  # one filler call per jt-pair per pr

                def filler(calls_left):
                    # spread chunk work evenly across the i-block's steps;
                    # p3 closures are held back for the normalize bubbles
                    if must:
                        calls_left = max(1, calls_left)
                        k = (len(must) + calls_left - 1) // calls_left
                        for _ in range(min(k, len(must))):
                            must.pop(0)()

                for pr in range(4):
                    ps_c0 = ctxps.tile([128, SB], F32, tag="ctx")
                    ps_c1 = ctxps.tile([128, SB], F32, tag="ctx")

                    def scores(jt):
                        r = jt - 4 * ib
                        f0 = 128 * r if r > 0 else 0
                        jsl = slice(jt * 128, (jt + 1) * 128)
                        qsl = slice(ib * SB + f0, (ib + 1) * SB)
                        ps_s = sp.tile([128, 2 * SB], F32, tag="s")
                        nc.tensor.matmul(
                            out=ps_s[:, f0:SB],
                            lhsT=kT[0:64, pr, jsl],
                            rhs=qT[0:64, pr, qsl],
                            start=True,
                            stop=True,
                        )
                        nc.tensor.matmul(
                            out=ps_s[:, SB + f0 : 2 * SB],
                            lhsT=kT[64:128, pr, jsl],
                            rhs=qT[64:128, pr, qsl],
                            start=True,
                            stop=True,
                        )
                        return ps_s

                    def softmax_ctx_pair(pair):
                        # exp both jts first, then ctx matmuls grouped by
                        # head so consecutive matmuls stay on one PSUM bank
                        exps = []
                        for jt, ps_s in pair:
                            r = jt - 4 * ib
                            f0 = 128 * r if r > 0 else 0
                            expT = expp.tile([128, 2 * SB], MMDT, tag="exp")
                            ps_v = ps_s[:].rearrange("p (t c) -> p t c", t=2)
                            ex_v = expT[:].rearrange("p (t c) -> p t c", t=2)
                            nc.scalar.activation(
                                out=ex_v[:, :, f0:SB],
                                in_=ps_v[:, :, f0:SB],
                                func=mybir.ActivationFunctionType.Exp,
                                scale=1.0 / np.sqrt(HD),
                            )
                            if r >= 0:
                                nc.vector.tensor_mul(
                                    ex_v[:, :, f0 : f0 + 128],
                                    ex_v[:, :, f0 : f0 + 128],
                                    masks_sb[:].unsqueeze(1).broadcast_to(
                                        (128, 2, 128)
                                    ),
                                )
                            exps.append((jt, f0, expT))
                        for t, ps_c in ((0, ps_c0), (1, ps_c1)):
                            coff = pr * 192 + (64 if t == 0 else 0)
                            for jt, f0, expT in exps:
                                nc.tensor.matmul(
                                    out=ps_c[:, f0:SB],
                                    lhsT=v_ext[:, jt, coff : coff + 128],
                                    rhs=expT[:, t * SB + f0 : (t + 1) * SB],
                                    start=(jt == 0),
                                    stop=(jt == njt - 1),
                                )

                    prev_pair = None
                    for base in range(0, njt, 2):
                        pair = [(jt, scores(jt)) for jt in (base, base + 1)]
                        if prev_pair is not None:
                            softmax_ctx_pair(prev_pair)
                        filler(steps_left)
                        steps_left -= 2
                        prev_pair = pair
                    softmax_ctx_pair(prev_pair)

                    # normalize.  ps_c0 is the bank the next pr's first ctx
                    # matmul will WAR-wait on, so copy it out whole and run
                    # the chain from SBUF to release the bank after one op.
                    # even head (ps_c0): denom rows 0:64, ctx rows 64:128
                    ce = smallp.tile([128, SB], F32, tag="ce")
                    nc.vector.tensor_copy(ce, ps_c0)
                    rdt0 = smallp.tile([128, SB], F32, tag="rdt0")
                    nc.vector.reciprocal_approx_fast(
                        rdt0[0:64, :], ce[0:64, :]
                    )
                    bce = smallp.tile([128, SB], F32, tag="bce")
                    nc.gpsimd.partition_broadcast(bce, rdt0[0:1, :])
                    nc.vector.tensor_mul(
                        ctxT[64:128, pr, isl], ce[64:128, :], bce[64:128, :]
                    )
                    # odd head (ps_c1): ctx rows 0:64, denom rows 64:128.
                    # recip only works at base partition 0, so shift the
                    # denominator down first with a partition-shifted copy.
                    dco = smallp.tile([128, SB], F32, tag="dco")
                    nc.vector.tensor_copy(dco[0:64, :], ps_c1[64:128, :])
                    rdt1 = smallp.tile([128, SB], F32, tag="rdt1")
                    nc.vector.reciprocal_approx_fast(
                        rdt1[0:64, :], dco[0:64, :]
                    )
                    nc.vector.tensor_mul(
                        ctxT[0:64, pr, isl], ps_c1[0:64, :], rdt1[0:64, :]
                    )

                    # fill the normalize bubble with held-back p3 work
                    for _ in range(2):
                        if opt:
                            opt.pop(0)()

                # chunk for the next i-block must be fully emitted before its
                # attention begins (PE executes in program order)
                for cl in must:
                    cl()
                must = []

            # tail: remaining output-projection tiles
            opt.extend(p3_closures(ib_order[-1]))
            for cl in opt:
                cl()

    nc.finalize()
    return nc


_NC = None


def _get_nc():
    global _NC
    if _NC is None:
        _NC = _build()
    return _NC


def _warr_mt(wT):
    # [D, EH] -> [128, NEG*NKG*128], mt-major to match the SBUF weight tile
    return np.ascontiguousarray(
        wT.reshape(NKG, 128, NEG, 128)
        .transpose(1, 2, 0, 3)
        .reshape(128, NEG * NKG * 128)
    )


def _warr(wT):
    # [D, EH] -> [128, NKG*EH] matching the SBUF weight tile layout
    return np.ascontiguousarray(
        wT.reshape(NKG, 128, EH).transpose(1, 0, 2).reshape(128, NKG * EH)
    )


def kernel(x, wq, wk, wv, wo, wo_b):
    global LAST_RESULT
    x = np.ascontiguousarray(np.asarray(x, dtype=np.float32))
    wq = np.asarray(wq, dtype=np.float32)
    wk = np.asarray(wk, dtype=np.float32)
    wv = np.asarray(wv, dtype=np.float32)
    wo = np.asarray(wo, dtype=np.float32)
    wo_b = np.asarray(wo_b, dtype=np.float32)

    pp, ff = np.ogrid[0:128, 0:128]
    masks = (pp <= ff).astype(np.float32)

    in_maps = []
    for c in range(NCORES):
        b, hh = c // 2, c % 2
        es = slice(hh * EH, (hh + 1) * EH)
        in_maps.append(
            {
                "xt": np.ascontiguousarray(
                    x[b].T.astype(MMNP)
                    .reshape(NKG, 128, NSB, SB)
                    .transpose(2, 0, 1, 3)
                ),
                "wqt": _warr_mt(wq[es, :].T.astype(MMNP)),
                "wkt": _warr_mt(wk[es, :].T.astype(MMNP)),
                "wvt": _warr(wv[es, :].T.astype(MMNP)),
                "wot": np.ascontiguousarray(
                    wo[:, es].T.astype(MMNP)
                    .reshape(4, 2, 64, D)[:, ::-1]
                    .reshape(NEG, 128, D)
                    .transpose(1, 0, 2)
                    .reshape(128, NEG * D)
                ),
                "masks": masks.astype(MMNP),
            }
        )

    nc = _get_nc()
    res = run_bass_kernel_spmd(nc, in_maps, list(range(NCORES)), trace=TRACE)
    LAST_RESULT = res

    out = np.empty((B, S, D), np.float32)
    for b in range(B):
        out[b] = res.results[2 * b]["out"] + res.results[2 * b + 1]["out"]
    out += wo_b[None, None, :]
    return out


# revision 30
# speedup vs baseline: 1.1540x; 1.1540x over previous
"""Multi-head causal attention on 8 Trainium2 NeuronCores.

Sharding: core c -> (batch b = c//2, head-half hh = c%2).  Each core computes
q/k/v projections for its 8 heads (column-sharded wq/wk/wv), causal attention,
and a full-width partial output projection (row-sharded wo).  Host sums the
two partials per batch and adds the bias.

Device-side layout trick: scores are computed transposed (scoresT[j, i]) so
that the softmax-weighted sum over keys (ctx) is a plain matmul with v as the
stationary operand.  Ones-columns baked alongside v produce the softmax
denominator replicated across 64 partitions in the same PSUM tile as ctx.

Scheduling: projection chunks (per 512-token s-block) and output-projection
tiles are emitted as PE "filler" interleaved into the attention jt-loop, so
the tensor engine stays busy while the scalar engine paces the exp chain.
The two heads of a pair run as row-tiled concurrent K=64 matmuls (partitions
0:64 / 64:128), so a score pair costs one 512-column pass.
"""

import numpy as np

import concourse.bass as bass
import concourse.mybir as mybir
import concourse.tile as tile
from concourse import bacc
from concourse.bass_utils import run_bass_kernel_spmd

# Problem shape (hardcoded; kernel.py must be self-contained).
B, S, D, H = 4, 2048, 1024, 16
HD = D // H           # 64 head dim
NCORES = 8
EH = D // 2           # 512: per-core e-width (8 heads)
NHL = H // 2          # 8 local heads per core
SB = 512              # s-block (free dim of most matmuls)
NSB = S // SB         # 4
NST = S // 128        # 16 s-tiles / j-tiles
NEG = EH // 128       # 4 e-groups of 128 partitions
NKG = D // 128        # 8 d-groups (contraction tiles)
VROW = 4 * 192        # v_ext row: 4x [v_even(64) | ones(64) | v_odd(64)] = 768

F32 = mybir.dt.float32
F32R = mybir.dt.float32r
BF16 = mybir.dt.bfloat16
MMDT = BF16          # dtype for matmul inputs (BF16 or F32R)
import ml_dtypes
MMNP = ml_dtypes.bfloat16 if MMDT == BF16 else np.float32

TRACE = False
LAST_RESULT = None


def _build():
    nc = bacc.Bacc()

    xT_d = nc.dram_tensor("xt", [NSB, 128, NKG * SB], MMDT, kind="ExternalInput")
    wqT_d = nc.dram_tensor("wqt", [128, NEG * NKG * 128], MMDT, kind="ExternalInput")
    wkT_d = nc.dram_tensor("wkt", [128, NEG * NKG * 128], MMDT, kind="ExternalInput")
    wvT_d = nc.dram_tensor("wvt", [128, NKG * EH], MMDT, kind="ExternalInput")
    woT_d = nc.dram_tensor("wot", [128, NEG * D], MMDT, kind="ExternalInput")
    masks_d = nc.dram_tensor("masks", [128, 128], MMDT, kind="ExternalInput")
    out_d = nc.dram_tensor("out", [S, D], F32, kind="ExternalOutput")

    with tile.TileContext(nc) as tc:
        with (
            tc.tile_pool(name="persist", bufs=1) as persist,
            tc.tile_pool(name="sharedp", bufs=2, space="PSUM") as sharedp,
            tc.tile_pool(name="ctxps", bufs=2, space="PSUM") as ctxps,
            tc.tile_pool(name="sp", bufs=2, space="PSUM") as sp,
            tc.tile_pool(name="expp", bufs=4) as expp,
            tc.tile_pool(name="smallp", bufs=5) as smallp,
            tc.tile_pool(name="p3", bufs=2) as p3,
        ):
            qT = persist.tile([128, NEG, S], MMDT)      # [e-part, e-group, s]
            kT = persist.tile([128, NEG, S], MMDT)
            v_ext = persist.tile([128, NST, VROW], MMDT)  # [s-part, s-tile, row]
            ctxT = persist.tile([128, NEG, S], MMDT)
            w_q = persist.tile([128, NEG, NKG, 128], MMDT)
            w_k = persist.tile([128, NEG, NKG, 128], MMDT)
            w_v = persist.tile([128, NKG, EH], MMDT)
            woT_sb = persist.tile([128, NEG, D], MMDT)
            masks_sb = persist.tile([128, 128], MMDT)
            xts_all = persist.tile([128, NSB, NKG, SB], MMDT)

            # input DMAs: host pre-arranges every tensor into the exact
            # SBUF layout, so each transfer is contiguous on both sides;
            # w_q/w_k/x are split finely so chunk-0 work starts early
            WCH = NKG * 128
            for mh in range(2):
                msl = slice(mh * 2, (mh + 1) * 2)
                nc.gpsimd.dma_start(
                    out=w_q[:, msl, :, :],
                    in_=wqT_d[:, mh * 2 * WCH : (mh + 1) * 2 * WCH].rearrange(
                        "p (m a e) -> p m a e", m=2, a=NKG
                    ),
                )
            for kh in range(2):
                ksl = slice(kh * (NKG // 2), (kh + 1) * (NKG // 2))
                csl = slice(kh * (NKG // 2) * SB, (kh + 1) * (NKG // 2) * SB)
                nc.sync.dma_start(
                    out=xts_all[:, 0, ksl, :],
                    in_=xT_d[0][:, csl].rearrange(
                        "p (a s) -> p a s", a=NKG // 2
                    ),
                )
            nc.gpsimd.dma_start(
                out=w_k,
                in_=wkT_d[:, :].rearrange("p (m a e) -> p m a e", m=NEG, a=NKG),
            )
            for sb in range(1, NSB):
                nc.sync.dma_start(
                    out=xts_all[:, sb, :, :],
                    in_=xT_d[sb].rearrange("p (a s) -> p a s", a=NKG),
                )
            nc.scalar.dma_start(
                out=w_v, in_=wvT_d[:, :].rearrange("p (a e) -> p a e", a=NKG)
            )
            nc.scalar.dma_start(out=masks_sb, in_=masks_d[:, :])
            nc.scalar.dma_start(
                out=woT_sb,
                in_=woT_d[:, :].rearrange("p (a e) -> p a e", a=NEG),
            )

            # shared ones block between each (even, odd) head pair
            for st in range(NST):
                for p in range(4):
                    ones_ap = v_ext[:, st, p * 192 + 64 : p * 192 + 128]
                    if MMDT == F32R:
                        ones_ap = ones_ap.bitcast(F32)
                    nc.vector.memset(ones_ap, 1.0)

            # ---------------- filler work generators ----------------
            def qk_closure(w_sb, dst, mt, sb):
                def go():
                    ssl = slice(sb * SB, (sb + 1) * SB)
                    msl = slice(mt * 128, (mt + 1) * 128)
                    ps = sharedp.tile([128, SB], F32, tag="acc")
                    for kg in range(NKG):
                        nc.tensor.matmul(
                            out=ps,
                            lhsT=(w_sb[:, mt, kg, :]),
                            rhs=(xts_all[:, sb, kg, :]),
                            start=(kg == 0),
                            stop=(kg == NKG - 1),
                        )
                    nc.vector.tensor_copy(dst[:, mt, ssl], ps)
                return go

            def v_closure(st4, sb):
                def go():
                    st = sb * (SB // 128) + st4
                    ps = sharedp.tile([128, EH], F32, tag="acc")
                    xsl = slice(st4 * 128, (st4 + 1) * 128)
                    for kg in range(NKG):
                        nc.tensor.matmul(
                            out=ps,
                            lhsT=(xts_all[:, sb, kg, xsl]),
                            rhs=(w_v[:, kg, :]),
                            start=(kg == 0),
                            stop=(kg == NKG - 1),
                        )
                    # psum cols: head h at [h*64, h*64+64); dest pair p:
                    # even head -> p*192+128, odd head -> p*192
                    psr = ps[:].rearrange("p (a c) -> p a c", c=128)
                    vst = v_ext[:, st, :].rearrange("p (a w) -> p a w", w=192)
                    nc.vector.tensor_copy(vst[:, :, 128:192], psr[:, :, 0:64])
                    nc.vector.tensor_copy(vst[:, :, 0:64], psr[:, :, 64:128])
                return go

            def chunk_closures(sb):
                cl = []
                for w_sb, dst in ((w_q, qT), (w_k, kT)):
                    for mt in range(NEG):
                        cl.append(qk_closure(w_sb, dst, mt, sb))
                for st4 in range(SB // 128):
                    cl.append(v_closure(st4, sb))
                return cl

            def p3_closure(it, ob):
                def go():
                    itsl = slice(it * 128, (it + 1) * 128)
                    osl = slice(ob * SB, (ob + 1) * SB)
                    ps = sharedp.tile([128, SB], F32, tag="acc")
                    for gg in range(NEG):
                        nc.tensor.matmul(
                            out=ps,
                            lhsT=(ctxT[:, gg, itsl]),
                            rhs=(woT_sb[:, gg, osl]),
                            start=(gg == 0),
                            stop=(gg == NEG - 1),
                        )
                    ot = p3.tile([128, SB], F32, tag="ot")
                    nc.vector.tensor_copy(ot, ps)
                    nc.sync.dma_start(out=out_d[itsl, osl], in_=ot)
                return go

            def p3_closures(ib):
                return [
                    p3_closure(it, ob)
                    for it in range(4 * ib, 4 * ib + 4)
                    for ob in range(2)
                ]

            # ---------------- attention with interleaved filler ----------------
            # i-block order [0,1,3,2]: the exp-heaviest block (3) runs while
            # p3 filler is still available, and the tail drains on the
            # smaller block 2 (whose own p3 is then the only serial tail)
            ib_order = [0, 1, 3, 2]
            must = list(chunk_closures(0))  # chunk 0 must precede attention 0
            for cl in must:
                cl()
            must = []
            opt = []
            chunks_done = {0}

            for idx, ib in enumerate(ib_order):
                isl = slice(ib * SB, (ib + 1) * SB)
                njt = 4 * (ib + 1)
                if idx + 1 < len(ib_order):
                    must = [
                        cl
                        for c in range(ib_order[idx + 1] + 1)
                        if c not in chunks_done
                        for cl in chunk_closures(c)
                    ]
                    chunks_done.update(range(ib_order[idx + 1] + 1))
                if idx >= 1:
                    opt.extend(p3_closures(ib_order[idx - 1]))
                steps_left = 2 * njt  # one filler call per jt-pair per pr

                def filler(calls_left):
                    # spread chunk work evenly across the i-block's steps;
                    # p3 closures are held back for the normalize bubbles
                    if must:
                        calls_left = max(1, calls_left)
                        k = (len(must) + calls_left - 1) // calls_left
                        for _ in range(min(k, len(must))):
                            must.pop(0)()

                for pr in range(4):
                    ps_c0 = ctxps.tile([128, SB], F32, tag="ctx")
                    ps_c1 = ctxps.tile([128, SB], F32, tag="ctx")

                    def scores(jt):
                        r = jt - 4 * ib
                        f0 = 128 * r if r > 0 else 0
                        jsl = slice(jt * 128, (jt + 1) * 128)
                        qsl = slice(ib * SB + f0, (ib + 1) * SB)
                        ps_s = sp.tile([128, 2 * SB], F32, tag="s")
                        nc.tensor.matmul(
                            out=ps_s[:, f0:SB],
                            lhsT=kT[0:64, pr, jsl],
                            rhs=qT[0:64, pr, qsl],
                            start=True,
                            stop=True,
                        )
                        nc.tensor.matmul(
                            out=ps_s[:, SB + f0 : 2 * SB],
                            lhsT=kT[64:128, pr, jsl],
                            rhs=qT[64:128, pr, qsl],
                            start=True,
                            stop=True,
                        )
                        return ps_s

                    def softmax_ctx_pair(pair):
                        # exp both jts first, then ctx matmuls grouped by
                        # head so consecutive matmuls stay on one PSUM bank
                        exps = []
                        for jt, ps_s in pair:
                            r = jt - 4 * ib
                            f0 = 128 * r if r > 0 else 0
                            expT = expp.tile([128, 2 * SB], MMDT, tag="exp")
                            ps_v = ps_s[:].rearrange("p (t c) -> p t c", t=2)
                            ex_v = expT[:].rearrange("p (t c) -> p t c", t=2)
                            nc.scalar.activation(
                                out=ex_v[:, :, f0:SB],
                                in_=ps_v[:, :, f0:SB],
                                func=mybir.ActivationFunctionType.Exp,
                                scale=1.0 / np.sqrt(HD),
                            )
                            if r >= 0:
                                nc.vector.tensor_mul(
                                    ex_v[:, :, f0 : f0 + 128],
                                    ex_v[:, :, f0 : f0 + 128],
                                    masks_sb[:].unsqueeze(1).broadcast_to(
                                        (128, 2, 128)
                                    ),
                                )
                            exps.append((jt, f0, expT))
                        for t, ps_c in ((0, ps_c0), (1, ps_c1)):
                            coff = pr * 192 + (64 if t == 0 else 0)
                            for jt, f0, expT in exps:
                                nc.tensor.matmul(
                                    out=ps_c[:, f0:SB],
                                    lhsT=v_ext[:, jt, coff : coff + 128],
                                    rhs=expT[:, t * SB + f0 : (t + 1) * SB],
                                    start=(jt == 0),
                                    stop=(jt == njt - 1),
                                )

                    prev_pair = None
                    for base in range(0, njt, 2):
                        pair = [(jt, scores(jt)) for jt in (base, base + 1)]
                        if prev_pair is not None:
                            softmax_ctx_pair(prev_pair)
                        filler(steps_left)
                        steps_left -= 2
                        prev_pair = pair
                    softmax_ctx_pair(prev_pair)

                    # normalize.  ps_c0 is the bank the next pr's first ctx
                    # matmul will WAR-wait on, so copy it out whole and run
                    # the chain from SBUF to release the bank after one op.
                    # even head (ps_c0): denom rows 0:64, ctx rows 64:128
                    ce = smallp.tile([128, SB], F32, tag="ce")
                    nc.vector.tensor_copy(ce, ps_c0)
                    rdt0 = smallp.tile([128, SB], F32, tag="rdt0")
                    nc.vector.reciprocal_approx_fast(
                        rdt0[0:64, :], ce[0:64, :]
                    )
                    bce = smallp.tile([128, SB], F32, tag="bce")
                    nc.gpsimd.partition_broadcast(bce, rdt0[0:1, :])
                    nc.vector.tensor_mul(
                        ctxT[64:128, pr, isl], ce[64:128, :], bce[64:128, :]
                    )
                    # odd head (ps_c1): ctx rows 0:64, denom rows 64:128.
                    # recip only works at base partition 0, so shift the
                    # denominator down first with a partition-shifted copy.
                    dco = smallp.tile([128, SB], F32, tag="dco")
                    nc.vector.tensor_copy(dco[0:64, :], ps_c1[64:128, :])
                    rdt1 = smallp.tile([128, SB], F32, tag="rdt1")
                    nc.vector.reciprocal_approx_fast(
                        rdt1[0:64, :], dco[0:64, :]
                    )
                    nc.vector.tensor_mul(
                        ctxT[0:64, pr, isl], ps_c1[0:64, :], rdt1[0:64, :]
                    )

                    # fill the normalize bubble with held-back p3 work
                    for _ in range(2):
                        if opt:
                            opt.pop(0)()

                # chunk for the next i-block must be fully emitted before its
                # attention begins (PE executes in program order)
                for cl in must:
                    cl()
                must = []

            # tail: remaining output-projection tiles
            opt.extend(p3_closures(ib_order[-1]))
            for cl in opt:
                cl()

    nc.finalize()
    return nc


_NC = None


def _get_nc():
    global _NC
    if _NC is None:
        _NC = _build()
    return _NC


def _warr_mt(wT):
    # [D, EH] -> [128, NEG*NKG*128], mt-major to match the SBUF weight tile
    return np.ascontiguousarray(
        wT.reshape(NKG, 128, NEG, 128)
        .transpose(1, 2, 0, 3)
        .reshape(128, NEG * NKG * 128)
    )


def _warr(wT):
    # [D, EH] -> [128, NKG*EH] matching the SBUF weight tile layout
    return np.ascontiguousarray(
        wT.reshape(NKG, 128, EH).transpose(1, 0, 2).reshape(128, NKG * EH)
    )


def kernel(x, wq, wk, wv, wo, wo_b):
    global LAST_RESULT
    x = np.ascontiguousarray(np.asarray(x, dtype=np.float32))
    wq = np.asarray(wq, dtype=np.float32)
    wk = np.asarray(wk, dtype=np.float32)
    wv = np.asarray(wv, dtype=np.float32)
    wo = np.asarray(wo, dtype=np.float32)
    wo_b = np.asarray(wo_b, dtype=np.float32)

    pp, ff = np.ogrid[0:128, 0:128]
    masks = (pp <= ff).astype(np.float32)

    in_maps = []
    for c in range(NCORES):
        b, hh = c // 2, c % 2
        es = slice(hh * EH, (hh + 1) * EH)
        in_maps.append(
            {
                "xt": np.ascontiguousarray(
                    x[b].T.astype(MMNP)
                    .reshape(NKG, 128, NSB, SB)
                    .transpose(2, 1, 0, 3)
                    .reshape(NSB, 128, NKG * SB)
                ),
                "wqt": _warr_mt(wq[es, :].T.astype(MMNP)),
                "wkt": _warr_mt(wk[es, :].T.astype(MMNP)),
                "wvt": _warr(wv[es, :].T.astype(MMNP)),
                "wot": np.ascontiguousarray(
                    wo[:, es].T.astype(MMNP)
                    .reshape(4, 2, 64, D)[:, ::-1]
                    .reshape(NEG, 128, D)
                    .transpose(1, 0, 2)
                    .reshape(128, NEG * D)
                ),
                "masks": masks.astype(MMNP),
            }
        )

    nc = _get_nc()
    res = run_bass_kernel_spmd(nc, in_maps, list(range(NCORES)), trace=TRACE)
    LAST_RESULT = res

    out = np.empty((B, S, D), np.float32)
    for b in range(B):
        out[b] = res.results[2 * b]["out"] + res.results[2 * b + 1]["out"]
    out += wo_b[None, None, :]
    return out


# revision 31
# speedup vs baseline: 1.1704x; 1.0142x over previous
"""Multi-head causal attention on 8 Trainium2 NeuronCores.

Sharding: core c -> (batch b = c//2, head-half hh = c%2).  Each core computes
q/k/v projections for its 8 heads (column-sharded wq/wk/wv), causal attention,
and a full-width partial output projection (row-sharded wo).  Host sums the
two partials per batch and adds the bias.

Device-side layout trick: scores are computed transposed (scoresT[j, i]) so
that the softmax-weighted sum over keys (ctx) is a plain matmul with v as the
stationary operand.  Ones-columns baked alongside v produce the softmax
denominator replicated across 64 partitions in the same PSUM tile as ctx.

Scheduling: projection chunks (per 512-token s-block) and output-projection
tiles are emitted as PE "filler" interleaved into the attention jt-loop, so
the tensor engine stays busy while the scalar engine paces the exp chain.
The two heads of a pair run as row-tiled concurrent K=64 matmuls (partitions
0:64 / 64:128), so a score pair costs one 512-column pass.
"""

import numpy as np

import concourse.bass as bass
import concourse.mybir as mybir
import concourse.tile as tile
from concourse import bacc
from concourse.bass_utils import run_bass_kernel_spmd

# Problem shape (hardcoded; kernel.py must be self-contained).
B, S, D, H = 4, 2048, 1024, 16
HD = D // H           # 64 head dim
NCORES = 8
EH = D // 2           # 512: per-core e-width (8 heads)
NHL = H // 2          # 8 local heads per core
SB = 512              # s-block (free dim of most matmuls)
NSB = S // SB         # 4
NST = S // 128        # 16 s-tiles / j-tiles
NEG = EH // 128       # 4 e-groups of 128 partitions
NKG = D // 128        # 8 d-groups (contraction tiles)
VROW = 4 * 192        # v_ext row: 4x [v_even(64) | ones(64) | v_odd(64)] = 768

F32 = mybir.dt.float32
F32R = mybir.dt.float32r
BF16 = mybir.dt.bfloat16
MMDT = BF16          # dtype for matmul inputs (BF16 or F32R)
import ml_dtypes
MMNP = ml_dtypes.bfloat16 if MMDT == BF16 else np.float32

TRACE = False
LAST_RESULT = None


def _build():
    nc = bacc.Bacc()

    xT_d = nc.dram_tensor("xt", [NSB, 128, NKG * SB], MMDT, kind="ExternalInput")
    wqT_d = nc.dram_tensor("wqt", [128, NEG * NKG * 128], MMDT, kind="ExternalInput")
    wkT_d = nc.dram_tensor("wkt", [128, NEG * NKG * 128], MMDT, kind="ExternalInput")
    wvT_d = nc.dram_tensor("wvt", [128, NKG * EH], MMDT, kind="ExternalInput")
    woT_d = nc.dram_tensor("wot", [128, NEG * D], MMDT, kind="ExternalInput")
    masks_d = nc.dram_tensor("masks", [128, 128], MMDT, kind="ExternalInput")
    out_d = nc.dram_tensor("out", [S, D], F32, kind="ExternalOutput")

    with tile.TileContext(nc) as tc:
        with (
            tc.tile_pool(name="persist", bufs=1) as persist,
            tc.tile_pool(name="sharedp", bufs=2, space="PSUM") as sharedp,
            tc.tile_pool(name="ctxps", bufs=2, space="PSUM") as ctxps,
            tc.tile_pool(name="sp", bufs=2, space="PSUM") as sp,
            tc.tile_pool(name="expp", bufs=4) as expp,
            tc.tile_pool(name="smallp", bufs=5) as smallp,
            tc.tile_pool(name="p3", bufs=2) as p3,
        ):
            qT = persist.tile([128, NEG, S], MMDT)      # [e-part, e-group, s]
            kT = persist.tile([128, NEG, S], MMDT)
            v_ext = persist.tile([128, NST, VROW], MMDT)  # [s-part, s-tile, row]
            ctxT = persist.tile([128, NEG, S], MMDT)
            w_q = persist.tile([128, NEG, NKG, 128], MMDT)
            w_k = persist.tile([128, NEG, NKG, 128], MMDT)
            w_v = persist.tile([128, NKG, EH], MMDT)
            woT_sb = persist.tile([128, NEG, D], MMDT)
            masks_sb = persist.tile([128, 128], MMDT)
            xts_all = persist.tile([128, NSB, NKG, SB], MMDT)

            # input DMAs: host pre-arranges every tensor into the exact
            # SBUF layout, so each transfer is contiguous on both sides;
            # w_q/w_k/x are split finely so chunk-0 work starts early
            nc.gpsimd.dma_start(
                out=w_q,
                in_=wqT_d[:, :].rearrange("p (m a e) -> p m a e", m=NEG, a=NKG),
            )
            for sb in range(NSB):
                nc.sync.dma_start(
                    out=xts_all[:, sb, :, :],
                    in_=xT_d[sb].rearrange("p (a s) -> p a s", a=NKG),
                )
            nc.gpsimd.dma_start(
                out=w_k,
                in_=wkT_d[:, :].rearrange("p (m a e) -> p m a e", m=NEG, a=NKG),
            )
            nc.scalar.dma_start(
                out=w_v, in_=wvT_d[:, :].rearrange("p (a e) -> p a e", a=NKG)
            )
            nc.scalar.dma_start(out=masks_sb, in_=masks_d[:, :])
            nc.scalar.dma_start(
                out=woT_sb,
                in_=woT_d[:, :].rearrange("p (a e) -> p a e", a=NEG),
            )

            # shared ones block between each (even, odd) head pair
            for st in range(NST):
                for p in range(4):
                    ones_ap = v_ext[:, st, p * 192 + 64 : p * 192 + 128]
                    if MMDT == F32R:
                        ones_ap = ones_ap.bitcast(F32)
                    nc.vector.memset(ones_ap, 1.0)

            # ---------------- filler work generators ----------------
            def qk_closure(w_sb, dst, mt, sb):
                def go():
                    ssl = slice(sb * SB, (sb + 1) * SB)
                    msl = slice(mt * 128, (mt + 1) * 128)
                    ps = sharedp.tile([128, SB], F32, tag="acc")
                    for kg in range(NKG):
                        nc.tensor.matmul(
                            out=ps,
                            lhsT=(w_sb[:, mt, kg, :]),
                            rhs=(xts_all[:, sb, kg, :]),
                            start=(kg == 0),
                            stop=(kg == NKG - 1),
                        )
                    nc.vector.tensor_copy(dst[:, mt, ssl], ps)
                return go

            def v_closure(st4, sb):
                def go():
                    st = sb * (SB // 128) + st4
                    ps = sharedp.tile([128, EH], F32, tag="acc")
                    xsl = slice(st4 * 128, (st4 + 1) * 128)
                    for kg in range(NKG):
                        nc.tensor.matmul(
                            out=ps,
                            lhsT=(xts_all[:, sb, kg, xsl]),
                            rhs=(w_v[:, kg, :]),
                            start=(kg == 0),
                            stop=(kg == NKG - 1),
                        )
                    # psum cols: head h at [h*64, h*64+64); dest pair p:
                    # even head -> p*192+128, odd head -> p*192
                    psr = ps[:].rearrange("p (a c) -> p a c", c=128)
                    vst = v_ext[:, st, :].rearrange("p (a w) -> p a w", w=192)
                    nc.vector.tensor_copy(vst[:, :, 128:192], psr[:, :, 0:64])
                    nc.vector.tensor_copy(vst[:, :, 0:64], psr[:, :, 64:128])
                return go

            def chunk_closures(sb):
                cl = []
                for w_sb, dst in ((w_q, qT), (w_k, kT)):
                    for mt in range(NEG):
                        cl.append(qk_closure(w_sb, dst, mt, sb))
                for st4 in range(SB // 128):
                    cl.append(v_closure(st4, sb))
                return cl

            def p3_closure(it, ob):
                def go():
                    itsl = slice(it * 128, (it + 1) * 128)
                    osl = slice(ob * SB, (ob + 1) * SB)
                    ps = sharedp.tile([128, SB], F32, tag="acc")
                    for gg in range(NEG):
                        nc.tensor.matmul(
                            out=ps,
                            lhsT=(ctxT[:, gg, itsl]),
                            rhs=(woT_sb[:, gg, osl]),
                            start=(gg == 0),
                            stop=(gg == NEG - 1),
                        )
                    ot = p3.tile([128, SB], F32, tag="ot")
                    nc.vector.tensor_copy(ot, ps)
                    nc.sync.dma_start(out=out_d[itsl, osl], in_=ot)
                return go

            def p3_closures(ib):
                return [
                    p3_closure(it, ob)
                    for it in range(4 * ib, 4 * ib + 4)
                    for ob in range(2)
                ]

            # ---------------- attention with interleaved filler ----------------
            # i-block order [0,1,3,2]: the exp-heaviest block (3) runs while
            # p3 filler is still available, and the tail drains on the
            # smaller block 2 (whose own p3 is then the only serial tail)
            ib_order = [0, 1, 3, 2]
            must = list(chunk_closures(0))  # chunk 0 must precede attention 0
            for cl in must:
                cl()
            must = []
            opt = []
            chunks_done = {0}

            for idx, ib in enumerate(ib_order):
                isl = slice(ib * SB, (ib + 1) * SB)
                njt = 4 * (ib + 1)
                if idx + 1 < len(ib_order):
                    must = [
                        cl
                        for c in range(ib_order[idx + 1] + 1)
                        if c not in chunks_done
                        for cl in chunk_closures(c)
                    ]
                    chunks_done.update(range(ib_order[idx + 1] + 1))
                if idx >= 1:
                    opt.extend(p3_closures(ib_order[idx - 1]))
                steps_left = 2 * njt  # one filler call per jt-pair per pr

                def filler(calls_left):
                    # spread chunk work evenly across the i-block's steps;
                    # p3 closures are held back for the normalize bubbles
                    if must:
                        calls_left = max(1, calls_left)
                        k = (len(must) + calls_left - 1) // calls_left
                        for _ in range(min(k, len(must))):
                            must.pop(0)()

                for pr in range(4):
                    ps_c0 = ctxps.tile([128, SB], F32, tag="ctx")
                    ps_c1 = ctxps.tile([128, SB], F32, tag="ctx")

                    def scores(jt):
                        r = jt - 4 * ib
                        f0 = 128 * r if r > 0 else 0
                        jsl = slice(jt * 128, (jt + 1) * 128)
                        qsl = slice(ib * SB + f0, (ib + 1) * SB)
                        ps_s = sp.tile([128, 2 * SB], F32, tag="s")
                        nc.tensor.matmul(
                            out=ps_s[:, f0:SB],
                            lhsT=kT[0:64, pr, jsl],
                            rhs=qT[0:64, pr, qsl],
                            start=True,
                            stop=True,
                        )
                        nc.tensor.matmul(
                            out=ps_s[:, SB + f0 : 2 * SB],
                            lhsT=kT[64:128, pr, jsl],
                            rhs=qT[64:128, pr, qsl],
                            start=True,
                            stop=True,
                        )
                        return ps_s

                    def softmax_ctx_pair(pair):
                        # exp both jts first, then ctx matmuls grouped by
                        # head so consecutive matmuls stay on one PSUM bank
                        exps = []
                        for jt, ps_s in pair:
                            r = jt - 4 * ib
                            f0 = 128 * r if r > 0 else 0
                            expT = expp.tile([128, 2 * SB], MMDT, tag="exp")
                            ps_v = ps_s[:].rearrange("p (t c) -> p t c", t=2)
                            ex_v = expT[:].rearrange("p (t c) -> p t c", t=2)
                            nc.scalar.activation(
                                out=ex_v[:, :, f0:SB],
                                in_=ps_v[:, :, f0:SB],
                                func=mybir.ActivationFunctionType.Exp,
                                scale=1.0 / np.sqrt(HD),
                            )
                            if r >= 0:
                                nc.vector.tensor_mul(
                                    ex_v[:, :, f0 : f0 + 128],
                                    ex_v[:, :, f0 : f0 + 128],
                                    masks_sb[:].unsqueeze(1).broadcast_to(
                                        (128, 2, 128)
                                    ),
                                )
                            exps.append((jt, f0, expT))
                        for t, ps_c in ((0, ps_c0), (1, ps_c1)):
                            coff = pr * 192 + (64 if t == 0 else 0)
                            for jt, f0, expT in exps:
                                nc.tensor.matmul(
                                    out=ps_c[:, f0:SB],
                                    lhsT=v_ext[:, jt, coff : coff + 128],
                                    rhs=expT[:, t * SB + f0 : (t + 1) * SB],
                                    start=(jt == 0),
                                    stop=(jt == njt - 1),
                                )

                    prev_pair = None
                    for base in range(0, njt, 2):
                        pair = [(jt, scores(jt)) for jt in (base, base + 1)]
                        if prev_pair is not None:
                            softmax_ctx_pair(prev_pair)
                        filler(steps_left)
                        steps_left -= 2
                        prev_pair = pair
                    softmax_ctx_pair(prev_pair)

                    # normalize.  ps_c0 is the bank the next pr's first ctx
                    # matmul will WAR-wait on, so copy it out whole and run
                    # the chain from SBUF to release the bank after one op.
                    # even head (ps_c0): denom rows 0:64, ctx rows 64:128
                    ce = smallp.tile([128, SB], F32, tag="ce")
                    nc.vector.tensor_copy(ce, ps_c0)
                    rdt0 = smallp.tile([128, SB], F32, tag="rdt0")
                    nc.vector.reciprocal_approx_fast(
                        rdt0[0:64, :], ce[0:64, :]
                    )
                    bce = smallp.tile([128, SB], F32, tag="bce")
                    nc.gpsimd.partition_broadcast(bce, rdt0[0:1, :])
                    nc.vector.tensor_mul(
                        ctxT[64:128, pr, isl], ce[64:128, :], bce[64:128, :]
                    )
                    # odd head (ps_c1): ctx rows 0:64, denom rows 64:128.
                    # recip only works at base partition 0, so shift the
                    # denominator down first with a partition-shifted copy.
                    dco = smallp.tile([128, SB], F32, tag="dco")
                    nc.vector.tensor_copy(dco[0:64, :], ps_c1[64:128, :])
                    rdt1 = smallp.tile([128, SB], F32, tag="rdt1")
                    nc.vector.reciprocal_approx_fast(
                        rdt1[0:64, :], dco[0:64, :]
                    )
                    nc.vector.tensor_mul(
                        ctxT[0:64, pr, isl], ps_c1[0:64, :], rdt1[0:64, :]
                    )

                    # fill the normalize bubble with held-back p3 work
                    for _ in range(2):
                        if opt:
                            opt.pop(0)()

                # chunk for the next i-block must be fully emitted before its
                # attention begins (PE executes in program order)
                for cl in must:
                    cl()
                must = []

            # tail: remaining output-projection tiles
            opt.extend(p3_closures(ib_order[-1]))
            for cl in opt:
                cl()

    nc.finalize()
    return nc


_NC = None


def _get_nc():
    global _NC
    if _NC is None:
        _NC = _build()
    return _NC


def _warr_mt(wT):
    # [D, EH] -> [128, NEG*NKG*128], mt-major to match the SBUF weight tile
    return np.ascontiguousarray(
        wT.reshape(NKG, 128, NEG, 128)
        .transpose(1, 2, 0, 3)
        .reshape(128, NEG * NKG * 128)
    )


def _warr(wT):
    # [D, EH] -> [128, NKG*EH] matching the SBUF weight tile layout
    return np.ascontiguousarray(
        wT.reshape(NKG, 128, EH).transpose(1, 0, 2).reshape(128, NKG * EH)
    )


def kernel(x, wq, wk, wv, wo, wo_b):
    global LAST_RESULT
    x = np.ascontiguousarray(np.asarray(x, dtype=np.float32))
    wq = np.asarray(wq, dtype=np.float32)
    wk = np.asarray(wk, dtype=np.float32)
    wv = np.asarray(wv, dtype=np.float32)
    wo = np.asarray(wo, dtype=np.float32)
    wo_b = np.asarray(wo_b, dtype=np.float32)

    pp, ff = np.ogrid[0:128, 0:128]
    masks = (pp <= ff).astype(np.float32)

    in_maps = []
    for c in range(NCORES):
        b, hh = c // 2, c % 2
        es = slice(hh * EH, (hh + 1) * EH)
        in_maps.append(
            {
                "xt": np.ascontiguousarray(
                    x[b].T.astype(MMNP)
                    .reshape(NKG, 128, NSB, SB)
                    .transpose(2, 1, 0, 3)
                    .reshape(NSB, 128, NKG * SB)
                ),
                "wqt": _warr_mt(wq[es, :].T.astype(MMNP)),
                "wkt": _warr_mt(wk[es, :].T.astype(MMNP)),
                "wvt": _warr(wv[es, :].T.astype(MMNP)),
                "wot": np.ascontiguousarray(
                    wo[:, es].T.astype(MMNP)
                    .reshape(4, 2, 64, D)[:, ::-1]
                    .reshape(NEG, 128, D)
                    .transpose(1, 0, 2)
                    .reshape(128, NEG * D)
                ),
                "masks": masks.astype(MMNP),
            }
        )

    nc = _get_nc()
    res = run_bass_kernel_spmd(nc, in_maps, list(range(NCORES)), trace=TRACE)
    LAST_RESULT = res

    out = np.empty((B, S, D), np.float32)
    for b in range(B):
        out[b] = res.results[2 * b]["out"] + res.results[2 * b + 1]["out"]
    out += wo_b[None, None, :]
    return out


# revision 32
# speedup vs baseline: 1.1889x; 1.0158x over previous
"""Multi-head causal attention on 8 Trainium2 NeuronCores.

Sharding: core c -> (batch b = c//2, head-half hh = c%2).  Each core computes
q/k/v projections for its 8 heads (column-sharded wq/wk/wv), causal attention,
and a full-width partial output projection (row-sharded wo).  Host sums the
two partials per batch and adds the bias.

Device-side layout trick: scores are computed transposed (scoresT[j, i]) so
that the softmax-weighted sum over keys (ctx) is a plain matmul with v as the
stationary operand.  Ones-columns baked alongside v produce the softmax
denominator replicated across 64 partitions in the same PSUM tile as ctx.

Scheduling: projection chunks (per 512-token s-block) and output-projection
tiles are emitted as PE "filler" interleaved into the attention jt-loop, so
the tensor engine stays busy while the scalar engine paces the exp chain.
The two heads of a pair run as row-tiled concurrent K=64 matmuls (partitions
0:64 / 64:128), so a score pair costs one 512-column pass.
"""

import numpy as np

import concourse.bass as bass
import concourse.mybir as mybir
import concourse.tile as tile
from concourse import bacc
from concourse.bass_utils import run_bass_kernel_spmd

# Problem shape (hardcoded; kernel.py must be self-contained).
B, S, D, H = 4, 2048, 1024, 16
HD = D // H           # 64 head dim
NCORES = 8
EH = D // 2           # 512: per-core e-width (8 heads)
NHL = H // 2          # 8 local heads per core
SB = 512              # s-block (free dim of most matmuls)
NSB = S // SB         # 4
NST = S // 128        # 16 s-tiles / j-tiles
NEG = EH // 128       # 4 e-groups of 128 partitions
NKG = D // 128        # 8 d-groups (contraction tiles)
VROW = 4 * 192        # v_ext row: 4x [v_even(64) | ones(64) | v_odd(64)] = 768

F32 = mybir.dt.float32
F32R = mybir.dt.float32r
BF16 = mybir.dt.bfloat16
MMDT = BF16          # dtype for matmul inputs (BF16 or F32R)
import ml_dtypes
MMNP = ml_dtypes.bfloat16 if MMDT == BF16 else np.float32

TRACE = False
LAST_RESULT = None


def _build():
    nc = bacc.Bacc()

    xT_d = nc.dram_tensor("xt", [NSB, 128, NKG * SB], MMDT, kind="ExternalInput")
    wqT_d = nc.dram_tensor("wqt", [128, NEG * NKG * 128], MMDT, kind="ExternalInput")
    wkT_d = nc.dram_tensor("wkt", [128, NEG * NKG * 128], MMDT, kind="ExternalInput")
    wvT_d = nc.dram_tensor("wvt", [128, NKG * EH], MMDT, kind="ExternalInput")
    woT_d = nc.dram_tensor("wot", [128, NEG * D], MMDT, kind="ExternalInput")
    masks_d = nc.dram_tensor("masks", [128, 128], MMDT, kind="ExternalInput")
    out_d = nc.dram_tensor("out", [S, D], BF16, kind="ExternalOutput")

    with tile.TileContext(nc) as tc:
        with (
            tc.tile_pool(name="persist", bufs=1) as persist,
            tc.tile_pool(name="sharedp", bufs=2, space="PSUM") as sharedp,
            tc.tile_pool(name="ctxps", bufs=2, space="PSUM") as ctxps,
            tc.tile_pool(name="sp", bufs=2, space="PSUM") as sp,
            tc.tile_pool(name="expp", bufs=4) as expp,
            tc.tile_pool(name="smallp", bufs=5) as smallp,
            tc.tile_pool(name="p3", bufs=2) as p3,
        ):
            qT = persist.tile([128, NEG, S], MMDT)      # [e-part, e-group, s]
            kT = persist.tile([128, NEG, S], MMDT)
            v_ext = persist.tile([128, NST, VROW], MMDT)  # [s-part, s-tile, row]
            ctxT = persist.tile([128, NEG, S], MMDT)
            w_q = persist.tile([128, NEG, NKG, 128], MMDT)
            w_k = persist.tile([128, NEG, NKG, 128], MMDT)
            w_v = persist.tile([128, NKG, EH], MMDT)
            woT_sb = persist.tile([128, NEG, D], MMDT)
            masks_sb = persist.tile([128, 128], MMDT)
            xts_all = persist.tile([128, NSB, NKG, SB], MMDT)

            # input DMAs: host pre-arranges every tensor into the exact
            # SBUF layout, so each transfer is contiguous on both sides;
            # w_q/w_k/x are split finely so chunk-0 work starts early
            nc.gpsimd.dma_start(
                out=w_q,
                in_=wqT_d[:, :].rearrange("p (m a e) -> p m a e", m=NEG, a=NKG),
            )
            for sb in range(NSB):
                nc.sync.dma_start(
                    out=xts_all[:, sb, :, :],
                    in_=xT_d[sb].rearrange("p (a s) -> p a s", a=NKG),
                )
            nc.gpsimd.dma_start(
                out=w_k,
                in_=wkT_d[:, :].rearrange("p (m a e) -> p m a e", m=NEG, a=NKG),
            )
            nc.scalar.dma_start(
                out=w_v, in_=wvT_d[:, :].rearrange("p (a e) -> p a e", a=NKG)
            )
            nc.scalar.dma_start(out=masks_sb, in_=masks_d[:, :])
            nc.scalar.dma_start(
                out=woT_sb,
                in_=woT_d[:, :].rearrange("p (a e) -> p a e", a=NEG),
            )

            # warmup: dummy matmuls on garbage SBUF keep the PE active (and
            # the HAM clock-gate open) while the input DMAs land; results are
            # discarded
            for wu in range(5):
                wps = sharedp.tile([128, SB], F32, tag="acc")
                for r in range(8):
                    nc.tensor.matmul(
                        out=wps,
                        lhsT=v_ext[:, 0, 0:128],
                        rhs=v_ext[:, 1, 0:SB],
                        start=(r == 0),
                        stop=(r == 7),
                    )

            # shared ones block between each (even, odd) head pair
            for st in range(NST):
                for p in range(4):
                    ones_ap = v_ext[:, st, p * 192 + 64 : p * 192 + 128]
                    if MMDT == F32R:
                        ones_ap = ones_ap.bitcast(F32)
                    nc.vector.memset(ones_ap, 1.0)

            # ---------------- filler work generators ----------------
            def qk_closure(w_sb, dst, mt, sb):
                def go():
                    ssl = slice(sb * SB, (sb + 1) * SB)
                    msl = slice(mt * 128, (mt + 1) * 128)
                    ps = sharedp.tile([128, SB], F32, tag="acc")
                    for kg in range(NKG):
                        nc.tensor.matmul(
                            out=ps,
                            lhsT=(w_sb[:, mt, kg, :]),
                            rhs=(xts_all[:, sb, kg, :]),
                            start=(kg == 0),
                            stop=(kg == NKG - 1),
                        )
                    nc.vector.tensor_copy(dst[:, mt, ssl], ps)
                return go

            def v_closure(st4, sb):
                def go():
                    st = sb * (SB // 128) + st4
                    ps = sharedp.tile([128, EH], F32, tag="acc")
                    xsl = slice(st4 * 128, (st4 + 1) * 128)
                    for kg in range(NKG):
                        nc.tensor.matmul(
                            out=ps,
                            lhsT=(xts_all[:, sb, kg, xsl]),
                            rhs=(w_v[:, kg, :]),
                            start=(kg == 0),
                            stop=(kg == NKG - 1),
                        )
                    # psum cols: head h at [h*64, h*64+64); dest pair p:
                    # even head -> p*192+128, odd head -> p*192
                    psr = ps[:].rearrange("p (a c) -> p a c", c=128)
                    vst = v_ext[:, st, :].rearrange("p (a w) -> p a w", w=192)
                    nc.vector.tensor_copy(vst[:, :, 128:192], psr[:, :, 0:64])
                    nc.vector.tensor_copy(vst[:, :, 0:64], psr[:, :, 64:128])
                return go

            def chunk_closures(sb):
                cl = []
                for w_sb, dst in ((w_q, qT), (w_k, kT)):
                    for mt in range(NEG):
                        cl.append(qk_closure(w_sb, dst, mt, sb))
                for st4 in range(SB // 128):
                    cl.append(v_closure(st4, sb))
                return cl

            def p3_closure(it, ob):
                def go():
                    itsl = slice(it * 128, (it + 1) * 128)
                    osl = slice(ob * SB, (ob + 1) * SB)
                    ps = sharedp.tile([128, SB], F32, tag="acc")
                    for gg in range(NEG):
                        nc.tensor.matmul(
                            out=ps,
                            lhsT=(ctxT[:, gg, itsl]),
                            rhs=(woT_sb[:, gg, osl]),
                            start=(gg == 0),
                            stop=(gg == NEG - 1),
                        )
                    ot = p3.tile([128, SB], BF16, tag="ot")
                    nc.vector.tensor_copy(ot, ps)
                    nc.sync.dma_start(out=out_d[itsl, osl], in_=ot)
                return go

            def p3_closures(ib):
                return [
                    p3_closure(it, ob)
                    for it in range(4 * ib, 4 * ib + 4)
                    for ob in range(2)
                ]

            # ---------------- attention with interleaved filler ----------------
            # i-block order [0,1,3,2]: the exp-heaviest block (3) runs while
            # p3 filler is still available, and the tail drains on the
            # smaller block 2 (whose own p3 is then the only serial tail)
            ib_order = [0, 1, 3, 2]
            must = list(chunk_closures(0))  # chunk 0 must precede attention 0
            for cl in must:
                cl()
            must = []
            opt = []
            chunks_done = {0}

            for idx, ib in enumerate(ib_order):
                isl = slice(ib * SB, (ib + 1) * SB)
                njt = 4 * (ib + 1)
                if idx + 1 < len(ib_order):
                    must = [
                        cl
                        for c in range(ib_order[idx + 1] + 1)
                        if c not in chunks_done
                        for cl in chunk_closures(c)
                    ]
                    chunks_done.update(range(ib_order[idx + 1] + 1))
                if idx >= 1:
                    opt.extend(p3_closures(ib_order[idx - 1]))
                steps_left = 2 * njt  # one filler call per jt-pair per pr

                def filler(calls_left):
                    # spread chunk work evenly across the i-block's steps;
                    # p3 closures are held back for the normalize bubbles
                    if must:
                        calls_left = max(1, calls_left)
                        k = (len(must) + calls_left - 1) // calls_left
                        for _ in range(min(k, len(must))):
                            must.pop(0)()

                for pr in range(4):
                    ps_c0 = ctxps.tile([128, SB], F32, tag="ctx")
                    ps_c1 = ctxps.tile([128, SB], F32, tag="ctx")

                    def scores(jt):
                        r = jt - 4 * ib
                        f0 = 128 * r if r > 0 else 0
                        jsl = slice(jt * 128, (jt + 1) * 128)
                        qsl = slice(ib * SB + f0, (ib + 1) * SB)
                        ps_s = sp.tile([128, 2 * SB], F32, tag="s")
                        nc.tensor.matmul(
                            out=ps_s[:, f0:SB],
                            lhsT=kT[0:64, pr, jsl],
                            rhs=qT[0:64, pr, qsl],
                            start=True,
                            stop=True,
                        )
                        nc.tensor.matmul(
                            out=ps_s[:, SB + f0 : 2 * SB],
                            lhsT=kT[64:128, pr, jsl],
                            rhs=qT[64:128, pr, qsl],
                            start=True,
                            stop=True,
                        )
                        return ps_s

                    def softmax_ctx_pair(pair):
                        # exp both jts first, then ctx matmuls grouped by
                        # head so consecutive matmuls stay on one PSUM bank
                        exps = []
                        for jt, ps_s in pair:
                            r = jt - 4 * ib
                            f0 = 128 * r if r > 0 else 0
                            expT = expp.tile([128, 2 * SB], MMDT, tag="exp")
                            ps_v = ps_s[:].rearrange("p (t c) -> p t c", t=2)
                            ex_v = expT[:].rearrange("p (t c) -> p t c", t=2)
                            nc.scalar.activation(
                                out=ex_v[:, :, f0:SB],
                                in_=ps_v[:, :, f0:SB],
                                func=mybir.ActivationFunctionType.Exp,
                                scale=1.0 / np.sqrt(HD),
                            )
                            if r >= 0:
                                nc.vector.tensor_mul(
                                    ex_v[:, :, f0 : f0 + 128],
                                    ex_v[:, :, f0 : f0 + 128],
                                    masks_sb[:].unsqueeze(1).broadcast_to(
                                        (128, 2, 128)
                                    ),
                                )
                            exps.append((jt, f0, expT))
                        for t, ps_c in ((0, ps_c0), (1, ps_c1)):
                            coff = pr * 192 + (64 if t == 0 else 0)
                            for jt, f0, expT in exps:
                                nc.tensor.matmul(
                                    out=ps_c[:, f0:SB],
                                    lhsT=v_ext[:, jt, coff : coff + 128],
                                    rhs=expT[:, t * SB + f0 : (t + 1) * SB],
                                    start=(jt == 0),
                                    stop=(jt == njt - 1),
                                )

                    prev_pair = None
                    for base in range(0, njt, 2):
                        pair = [(jt, scores(jt)) for jt in (base, base + 1)]
                        if prev_pair is not None:
                            softmax_ctx_pair(prev_pair)
                        filler(steps_left)
                        steps_left -= 2
                        prev_pair = pair
                    softmax_ctx_pair(prev_pair)

                    # normalize.  ps_c0 is the bank the next pr's first ctx
                    # matmul will WAR-wait on, so copy it out whole and run
                    # the chain from SBUF to release the bank after one op.
                    # even head (ps_c0): denom rows 0:64, ctx rows 64:128
                    ce = smallp.tile([128, SB], F32, tag="ce")
                    nc.vector.tensor_copy(ce, ps_c0)
                    rdt0 = smallp.tile([128, SB], F32, tag="rdt0")
                    nc.vector.reciprocal_approx_fast(
                        rdt0[0:64, :], ce[0:64, :]
                    )
                    bce = smallp.tile([128, SB], F32, tag="bce")
                    nc.gpsimd.partition_broadcast(bce, rdt0[0:1, :])
                    nc.vector.tensor_mul(
                        ctxT[64:128, pr, isl], ce[64:128, :], bce[64:128, :]
                    )
                    # odd head (ps_c1): ctx rows 0:64, denom rows 64:128.
                    # recip only works at base partition 0, so shift the
                    # denominator down first with a partition-shifted copy.
                    dco = smallp.tile([128, SB], F32, tag="dco")
                    nc.vector.tensor_copy(dco[0:64, :], ps_c1[64:128, :])
                    rdt1 = smallp.tile([128, SB], F32, tag="rdt1")
                    nc.vector.reciprocal_approx_fast(
                        rdt1[0:64, :], dco[0:64, :]
                    )
                    nc.vector.tensor_mul(
                        ctxT[0:64, pr, isl], ps_c1[0:64, :], rdt1[0:64, :]
                    )

                    # fill the normalize bubble with held-back p3 work
                    for _ in range(2):
                        if opt:
                            opt.pop(0)()

                # chunk for the next i-block must be fully emitted before its
                # attention begins (PE executes in program order)
                for cl in must:
                    cl()
                must = []

            # tail: remaining output-projection tiles
            opt.extend(p3_closures(ib_order[-1]))
            for cl in opt:
                cl()

    nc.finalize()
    return nc


_NC = None


def _get_nc():
    global _NC
    if _NC is None:
        _NC = _build()
    return _NC


def _warr_mt(wT):
    # [D, EH] -> [128, NEG*NKG*128], mt-major to match the SBUF weight tile
    return np.ascontiguousarray(
        wT.reshape(NKG, 128, NEG, 128)
        .transpose(1, 2, 0, 3)
        .reshape(128, NEG * NKG * 128)
    )


def _warr(wT):
    # [D, EH] -> [128, NKG*EH] matching the SBUF weight tile layout
    return np.ascontiguousarray(
        wT.reshape(NKG, 128, EH).transpose(1, 0, 2).reshape(128, NKG * EH)
    )


def kernel(x, wq, wk, wv, wo, wo_b):
    global LAST_RESULT
    x = np.ascontiguousarray(np.asarray(x, dtype=np.float32))
    wq = np.asarray(wq, dtype=np.float32)
    wk = np.asarray(wk, dtype=np.float32)
    wv = np.asarray(wv, dtype=np.float32)
    wo = np.asarray(wo, dtype=np.float32)
    wo_b = np.asarray(wo_b, dtype=np.float32)

    pp, ff = np.ogrid[0:128, 0:128]
    masks = (pp <= ff).astype(np.float32)

    in_maps = []
    for c in range(NCORES):
        b, hh = c // 2, c % 2
        es = slice(hh * EH, (hh + 1) * EH)
        in_maps.append(
            {
                "xt": np.ascontiguousarray(
                    x[b].T.astype(MMNP)
                    .reshape(NKG, 128, NSB, SB)
                    .transpose(2, 1, 0, 3)
                    .reshape(NSB, 128, NKG * SB)
                ),
                "wqt": _warr_mt(wq[es, :].T.astype(MMNP)),
                "wkt": _warr_mt(wk[es, :].T.astype(MMNP)),
                "wvt": _warr(wv[es, :].T.astype(MMNP)),
                "wot": np.ascontiguousarray(
                    wo[:, es].T.astype(MMNP)
                    .reshape(4, 2, 64, D)[:, ::-1]
                    .reshape(NEG, 128, D)
                    .transpose(1, 0, 2)
                    .reshape(128, NEG * D)
                ),
                "masks": masks.astype(MMNP),
            }
        )

    nc = _get_nc()
    res = run_bass_kernel_spmd(nc, in_maps, list(range(NCORES)), trace=TRACE)
    LAST_RESULT = res

    out = np.empty((B, S, D), np.float32)
    for b in range(B):
        out[b] = res.results[2 * b]["out"].astype(np.float32) + res.results[
            2 * b + 1
        ]["out"].astype(np.float32)
    out += wo_b[None, None, :]
    return out
